# revision 1
# baseline (speedup 1.0000x reference)
"""Trainium2 Bass kernel for nn_CT_loss (data-parallel over batch, 8 cores).

Math (R is a general 3x3 matrix, not orthogonal):
  u   = A P0 + b0          A = R diag(e), b0 = t - 0.5 R e      (per batch)
  c   = G P0 + g0          G = R^T A,     g0 = R^T b0
  v_a = A[:,c1] Qa' + A[:,c2] Qb' + h_a  (Q' = Q-0.5), s = R^T t
  d_a = s_a u - c_a v_a ;  la = sqrt(|d_a|^2 m_a)
  loss = sum_a [sum(m_a) >= 3B] sum(la) / max(sum_a sum(m_a), 1)

Device trick 1: fold 1/s_a into v's affine coefficients (vt = v/s_a), so
  d~_a = u - c_a vt_a  is scalar-free; host multiplies the per-batch
  partial sums by |s_a| during the gather.
Device trick 2: avoid scalar_tensor_tensor entirely (no fast DVE uop, 1x):
  every op is tensor_scalar/activation (1-src affine, 2x/1x) or
  tensor_tensor (2x bf16) combining them.

Layout per core: 8 batches; tiles [128, FD=1024]; partition = b*16+g,
free = 1024 pixels. Per-batch scalars ride as per-partition [128,1]
columns of a constants tile. Free-dim sums via accum_out; host finishes
the 128-row + cross-core reduction (the "gather").
"""
import os
import sys

import numpy as np

for _p in ("/opt/trn_rl_repo",):
    if _p not in sys.path:
        sys.path.insert(0, _p)

import concourse.bass as bass
import concourse.bacc as bacc
import concourse.tile as tile
from concourse import mybir
from concourse.bass_utils import run_bass_kernel_spmd

from ml_dtypes import bfloat16

F32 = mybir.dt.float32
BF16 = mybir.dt.bfloat16
AF = mybir.ActivationFunctionType
OP = mybir.AluOpType

B, HW = 64, 128 * 128
NCORES, BPC, G, FD = 8, 8, 16, 1024
F3 = 3 * FD

# a -> (Acol1, Acol2, qchA, qchB)
QCH = {0: (1, 2, 0, 1), 1: (0, 2, 2, 3), 2: (0, 1, 4, 5)}

# constants tile columns
CA = 0    # A[i*3+j] 9
CB0 = 9   # b0 3
CG = 12   # G[a*3+j] 9
CG0 = 21  # g0 3
CV1 = 24  # alpha~[a*3+i] = A[i,c1]/s~_a 9
CHC = 33  # h~[a*3+i] 9
CV2 = 42  # beta~[a*3+i] = A[i,c2]/s~_a 9
CZ = 51   # 0.0 (zero bias so ACT terms can always use Identity)
NCST = 52

# engine for each 1-src scaled-term family: u terms, c terms, v1, v2
E_TERMS = {"u": "act", "c": "act", "v1": "act", "v2": "vec"}
E_SQ_A = ["act", "act", "vec"]  # squares engine per a
E_MSUM = "act"    # mask sums via activation accum
E_W = "vec"       # w = sq0+sq1+sq2
E_WM = "vec"      # w *= mask

_BUILT = None
LAST = None


def _term(nc, eng, out, in_, sc, bi):
    """out = in_*sc + bi, per-partition scalar APs (bi required)."""
    if eng == "act":
        nc.scalar.activation(out, in_, AF.Identity, bias=bi, scale=sc)
    else:
        e = nc.vector if eng == "vec" else nc.gpsimd
        e.tensor_scalar(out, in_, sc, bi, op0=OP.mult, op1=OP.add)


def _eng(nc, eng):
    return nc.vector if eng == "vec" else nc.gpsimd


def _bcast3(ap, n):
    """[128, FD] AP -> [128, n, FD] with step-0 middle dim."""
    return bass.AP(tensor=ap.tensor, offset=ap.offset,
                   ap=[ap.ap[0], [0, n], *ap.ap[1:]])


def _build_nc():
    nc = bacc.Bacc(None)
    p0 = nc.dram_tensor("p0", [BPC, G, 3, FD], BF16, kind="ExternalInput")
    q0 = nc.dram_tensor("q0", [BPC, G, 6, FD], BF16, kind="ExternalInput")
    mk = nc.dram_tensor("mk", [BPC, G, 3, FD], BF16, kind="ExternalInput")
    cst = nc.dram_tensor("cst", [128, NCST], F32, kind="ExternalInput")
    outp = nc.dram_tensor("out", [128, 6], F32, kind="ExternalOutput")

    with tile.TileContext(nc) as tc:
        with tc.tile_pool(name="main", bufs=1) as pool, \
             tc.tile_pool(name="terms", bufs=6) as terms:
            # two HWDGE rings: sync gets p0+mk, scalar gets cst+q0
            cst_t = pool.tile([128, NCST], F32, tag="cst")
            nc.scalar.dma_start(cst_t[:], cst[:])

            def cs(j):
                return cst_t[:, j:j + 1]

            warm = pool.tile([128, 1], BF16, tag="warm")
            nc.scalar.activation(warm[:], cst_t[:, CZ:CZ + 1], AF.Sqrt)

            p0_t = pool.tile([128, 3, FD], BF16, tag="p0")
            p0r = p0[:].rearrange("b g c f -> (b g) c f")
            nc.sync.dma_start(p0_t[:, 0:2, :], p0r[:, 0:2, :])
            nc.scalar.dma_start(p0_t[:, 2:3, :], p0r[:, 2:3, :])
            q0_t = pool.tile([128, 6, FD], BF16, tag="q0")
            q0r = q0[:].rearrange("b g c f -> (b g) c f")
            for cc in range(3):
                nc.scalar.dma_start(q0_t[:, 2 * cc:2 * cc + 2, :],
                                    q0r[:, 2 * cc:2 * cc + 2, :])
            mk_t = pool.tile([128, 3, FD], BF16, tag="mk")
            nc.sync.dma_start(mk_t[:], mk[:].rearrange("b g c f -> (b g) c f"))

            acc = pool.tile([128, 6], F32, tag="acc")

            X = [p0_t[:, j, :] for j in range(3)]
            Q = [q0_t[:, j, :] for j in range(6)]
            MSK = [mk_t[:, a, :] for a in range(3)]

            zero = cs(CZ)

            def lin3(eng, outs, srcs, csc, cbi):
                for k in range(3):
                    t2 = terms.tile([128, FD], BF16, name="t2x", tag="t2")
                    _term(nc, eng[0], t2, srcs[2], csc(k, 2), cbi(k))
                    t1 = terms.tile([128, FD], BF16, name="t1x", tag="t1")
                    _term(nc, eng[1], t1, srcs[1], csc(k, 1), zero)
                    t0 = terms.tile([128, FD], BF16, name="t0x", tag="t0")
                    _term(nc, eng[2], t0, srcs[0], csc(k, 0), zero)
                    nc.vector.tensor_add(outs[k], t2, t1)
                    nc.vector.tensor_add(outs[k], outs[k], t0)

            u3 = pool.tile([128, 3, FD], BF16, tag="u3")
            lin3(["act", "vec", "act"], [u3[:, i, :] for i in range(3)],
                 X, lambda i, j: cs(CA + 3 * i + j), lambda i: cs(CB0 + i))
            c3 = pool.tile([128, 3, FD], BF16, tag="c3")
            lin3(["vec", "act", "vec"], [c3[:, a, :] for a in range(3)],
                 X, lambda a, j: cs(CG + 3 * a + j), lambda a: cs(CG0 + a))
            c_t = [c3[:, a, :] for a in range(3)]

            # mask sums early on ACT (fills ramp idle; accum -> host)
            scr = pool.tile([128, FD], BF16, tag="scr")
            for a in range(3):
                nc.scalar.activation(scr, MSK[a], AF.Identity, bias=zero,
                                     accum_out=acc[:, 3 + a:4 + a])

            vas = []
            for a in range(3):
                c1, c2, qA, qB = QCH[a]
                va = pool.tile([128, 3, FD], BF16, name=f"va{a}", tag=f"va{a}")
                vas.append(va)
                for i in range(3):
                    tv1 = terms.tile([128, FD], BF16, name="tv1x", tag="tv1")
                    _term(nc, E_TERMS["v1"], tv1, Q[qA], cs(CV1 + 3 * a + i),
                          cs(CHC + 3 * a + i))
                    tv2 = terms.tile([128, FD], BF16, name="tv2x", tag="tv2")
                    _term(nc, E_TERMS["v2"], tv2, Q[qB], cs(CV2 + 3 * a + i),
                          zero)
                    nc.vector.tensor_add(va[:, i, :], tv1, tv2)
                nc.vector.tensor_mul(va[:], _bcast3(c_t[a], 3), va[:])
                nc.vector.tensor_sub(va[:], u3[:], va[:])
                sq = pool.tile([128, 3, FD], BF16, name=f"sq{a}", tag=f"sq{a}")
                if E_SQ_A[a] == "act":
                    nc.scalar.activation(sq[:], va[:], AF.Square)
                else:
                    nc.vector.tensor_mul(sq[:], va[:], va[:])
                vas[a] = sq
            for a in range(3):
                sq = vas[a]
                w = pool.tile([128, FD], BF16, name=f"w{a}", tag=f"w{a}")
                nc.vector.tensor_add(w, sq[:, 0, :], sq[:, 1, :])
                nc.vector.tensor_add(w, w, sq[:, 2, :])
                nc.vector.tensor_mul(w, w, MSK[a])
                la = pool.tile([128, FD], BF16, name=f"la{a}", tag="la")
                nc.scalar.activation(la, w, AF.Sqrt, accum_out=acc[:, a:a + 1])

            nc.sync.dma_start(outp[:], acc[:])

    nc.compile()
    return nc


def get_nc():
    global _BUILT
    if _BUILT is None:
        _BUILT = _build_nc()
    return _BUILT


def host_constants(R, T, E):
    """[B, NCST] fp32 constants (fp64 host math) + [B,3] |s| scales."""
    Bn = R.shape[0]
    out = np.zeros((Bn, NCST), np.float64)
    sabs = np.zeros((Bn, 3), np.float64)
    for b in range(Bn):
        Rb = R[b].astype(np.float64)
        tb = T[b].astype(np.float64)
        eb = E[b].astype(np.float64)
        A = Rb * eb[None, :]
        b0 = tb - 0.5 * (Rb @ eb)
        Gm = Rb.T @ A
        g0 = Rb.T @ b0
        s = Rb.T @ tb
        out[b, CA:CA + 9] = A.reshape(-1)
        out[b, CB0:CB0 + 3] = b0
        out[b, CG:CG + 9] = Gm.reshape(-1)
        out[b, CG0:CG0 + 3] = g0
        for a, (c1, c2, _, _) in QCH.items():
            sh = np.sign(s[a]) * max(abs(s[a]), 1e-12) if s[a] != 0 else 1e-12
            sabs[b, a] = abs(s[a])
            h = tb - 0.5 * (A[:, c1] + A[:, c2])
            out[b, CV1 + 3 * a:CV1 + 3 * a + 3] = A[:, c1] / sh
            out[b, CV2 + 3 * a:CV2 + 3 * a + 3] = A[:, c2] / sh
            out[b, CHC + 3 * a:CHC + 3 * a + 3] = h / sh
    return out.astype(np.float32), sabs


def make_in_maps(P0, Q0, M, cst):
    in_maps = []
    for k in range(NCORES):
        sl = slice(k * BPC, (k + 1) * BPC)
        in_maps.append({
            "p0": P0[sl].reshape(BPC, 3, G, FD).transpose(0, 2, 1, 3).astype(bfloat16),
            "q0": Q0[sl].reshape(BPC, 6, G, FD).transpose(0, 2, 1, 3).astype(bfloat16),
            "mk": M[sl].reshape(BPC, 3, G, FD).transpose(0, 2, 1, 3).astype(bfloat16),
            "cst": np.ascontiguousarray(np.repeat(cst[sl], G, axis=0)),
        })
    return in_maps


def kernel(pred_rots, pred_P0, pred_Q0, gt_occmask, roi_extent, pred_transes):
    global LAST
    R = np.asarray(pred_rots, np.float32)
    P0 = np.asarray(pred_P0, np.float32)
    Q0 = np.asarray(pred_Q0, np.float32)
    M = np.asarray(gt_occmask, np.float32)
    E = np.asarray(roi_extent, np.float32)
    T = np.asarray(pred_transes, np.float32)

    nc = get_nc()
    cst, sabs = host_constants(R, T, E)
    in_maps = make_in_maps(P0, Q0, M, cst)
    trace = os.environ.get("KERNEL_TRACE", "0") == "1"
    LAST = run_bass_kernel_spmd(nc, in_maps, core_ids=list(range(NCORES)),
                                trace=trace)
    S_a = np.zeros(3, np.float64)
    M_a = np.zeros(3, np.float64)
    for k, r in enumerate(LAST.results):
        o = r["out"].astype(np.float64)          # [128, 6]
        st = o[:, 0:3].reshape(BPC, G, 3).sum(axis=1)   # [BPC, 3] per-batch
        S_a += (st * sabs[k * BPC:(k + 1) * BPC]).sum(axis=0)
        M_a += o[:, 3:6].sum(axis=0)
    loss = sum(0.0 if M_a[a] < 3 * B else S_a[a] for a in range(3))
    total = max(M_a.sum(), 1.0)
    return np.asarray(np.float32(loss / total))

